# revision 31
# baseline (speedup 1.0000x reference)
"""Trainium2 Bass kernel for GQA attention (nn_Attention_15015205667492).

Reference computation (per batch b, seq s=2048, d=2048):
  q = (x @ wq)  -> 32 heads x 64     (RoPE)
  k = (x @ wk)  ->  8 kv heads x 64  (RoPE)
  v = (x @ wv)  ->  8 kv heads x 64
  causal softmax(q k^T / 8) @ v  (GQA: kv head = q head // 4)
  out = attn @ wo
Sharding (8 cores): DP2 x TP4.
  core c: batch = c//4, head-group g = c%4 (Q heads 8g..8g+7, KV heads 2g, 2g+1).
  Per-qb attention output (head-transposed, bf16) is AllGathered within the
  4-core batch group; each core then computes a 512-column slice of o_proj so
  per-core outputs are disjoint blocks of the final output.

Schedule: 4 rounds, one per 512-column sequence chunk.  Round c streams x
stage c+1 (Pool-queue DMA, 25ns issue vs 565ns on SP), projects K/V/Q for
chunk c, runs attention for query block c, and interleaves o_proj matmuls of
block c-2 into the attention stream (fills PE gaps while ScalarE computes
exp; AV matmuls lag QK by 2 blocks to hide the exp latency).  V is projected
transposed and flipped per 128-block with PE transpose matmuls (~5x fewer PE
instructions than the natural-layout projection).  Scores stay transposed
(S^T[k,q]); the AV psum's 65th row (ones column in V) accumulates softmax
denominators, inverted with reciprocal_approx_fast and spread with a gpsimd
partition_broadcast.  Drains run on Pool/ScalarE to keep VectorE for
masks+RoPE; output is written bf16 and widened on the host.
"""

import sys

sys.path.insert(0, "/opt/trn_rl_repo")

import numpy as np
import ml_dtypes

N_CORES = 8
H, KVH, HD = 32, 8, 64
RG = [[0, 1, 2, 3], [4, 5, 6, 7]]

_cache = {}


def build_program(S=2048, D=2048, enable_asserts=False, NO_CC=False, bench_iters=0,
                  phases=None, ablate=()):
    import concourse.mybir as mybir
    import concourse.tile as tile
    from concourse import bacc

    f32 = mybir.dt.float32
    bf16 = mybir.dt.bfloat16
    Exp = mybir.ActivationFunctionType.Exp

    DC = D // 128         # contraction chunks for projections (16)
    CS = 512              # seq chunk = query block
    NCH = S // CS         # rounds (4)
    DOUT = D // 4         # output column slice per core (512)
    HC = (H * HD) // 128  # o_proj contraction chunks (16)
    KB = S // 128         # key blocks (16)

    nc = bacc.Bacc(
        "TRN2",
        target_bir_lowering=False,
        debug=False,
        enable_asserts=enable_asserts,
        num_devices=N_CORES,
    )

    xT_d = nc.dram_tensor("xT", [D, S], bf16, kind="ExternalInput")
    wk_d = nc.dram_tensor("wkt", [128, DC * 128], bf16, kind="ExternalInput")
    wv_d = nc.dram_tensor("wvt", [128, DC * 128], bf16, kind="ExternalInput")
    wq_d = nc.dram_tensor("wqt", [128, DC * 512], bf16, kind="ExternalInput")
    wo_d = nc.dram_tensor("wot", [128, HC * DOUT], bf16, kind="ExternalInput")
    cos_d = nc.dram_tensor("cos2", [128, S], bf16, kind="ExternalInput")
    sin_d = nc.dram_tensor("sinsw2", [128, S], bf16, kind="ExternalInput")
    rot_d = nc.dram_tensor("rot", [128, 128], bf16, kind="ExternalInput")
    idn_d = nc.dram_tensor("ident", [128, 128], bf16, kind="ExternalInput")
    msk_d = nc.dram_tensor("masks", [128, 4, 1024], bf16, kind="ExternalInput")
    out_d = nc.dram_tensor("out", [S, DOUT], bf16, kind="ExternalOutput")

    with tile.TileContext(nc) as tc:
        with (
            tc.tile_pool(name="const", bufs=1) as const,
            tc.tile_pool(name="stream", bufs=2) as stream,
            tc.tile_pool(name="work", bufs=2) as work,
            tc.tile_pool(name="psA", bufs=2, space="PSUM") as psA,
            tc.tile_pool(name="psAV", bufs=1, space="PSUM") as psAV,
            tc.tile_pool(name="psO", bufs=1, space="PSUM") as psO,
            tc.tile_pool(name="psT", bufs=1, space="PSUM") as psT,
            tc.tile_pool(name="dram", bufs=1, space="DRAM") as dram,
        ):
            # ---------------- constants (order = SP-queue DMA order) --------
            wk_sb = const.tile([128, DC, 128], bf16, name="wk", tag="wk")
            nc.sync.dma_start(out=wk_sb[:], in_=wk_d[:, :])
            wv_sb = const.tile([128, DC, 128], bf16, name="wv", tag="wv")
            cos_sb = const.tile([128, S], bf16, name="cos", tag="cos")
            sin_sb = const.tile([128, S], bf16, name="sin", tag="sin")
            rot_sb = const.tile([128, 128], bf16, name="rot", tag="rot")
            idn_sb = const.tile([128, 128], bf16, name="idn", tag="idn")
            wq_sb = const.tile([128, DC, 512], bf16, name="wq", tag="wq")
            msk_sb = const.tile([128, 4, 1024], bf16, name="msk", tag="msk")
            wo_sb = const.tile([128, HC, DOUT], bf16, name="wo", tag="wo")

            ones_sb = const.tile([65, 64], bf16, name="ones", tag="ones")
            nc.vector.memset(ones_sb[:], 1.0)
            kTd = [
                const.tile([128, S], bf16, name=f"kTd{h}", tag=f"kTd{h}")
                for h in range(2)
            ]
            v_sb = []
            for kb in range(KB):
                vt = const.tile([128, 132], bf16, name=f"v{kb}", tag=f"v{kb}")
                nc.vector.memset(vt[:, 64:65], 1.0)
                nc.vector.memset(vt[:, 129:130], 1.0)
                v_sb.append(vt)

            # per-(block, pair) collective buffers: pair p's AllGather output
            # rows 128*g hold global head-pair hc = 4*g + p
            cc_in = [
                [dram.tile([128, 512], bf16, name=f"cin{qb}_{p}",
                           tag=f"cin{qb}_{p}") for p in range(4)]
                for qb in range(NCH)
            ]
            cc_out = [
                [dram.tile([512, 512], bf16, name=f"cout{qb}_{p}",
                           tag=f"cout{qb}_{p}") for p in range(4)]
                for qb in range(NCH)
            ]

            def emit_body():
                xts = [None] * NCH
                pstate = {"n": 0, "tile": None}

                def proj_ps():
                    h = pstate["n"] % 2
                    if h == 0:
                        pstate["tile"] = psA.tile([128, 1024], f32, name="pjp",
                                                  tag="ps")
                    pstate["n"] += 1
                    return pstate["tile"][:, 512 * h: 512 * (h + 1)]

                def load_stage(st, split=1):
                    t = stream.tile([128, DC, CS], bf16, name="xs", tag="xs",
                                    bufs=2)
                    q = DC // split
                    for i in range(split):
                        nc.sync.dma_start(
                            out=t[:, q * i: q * (i + 1), :],
                            in_=xT_d[128 * q * i: 128 * q * (i + 1),
                                     CS * st: CS * (st + 1)].rearrange(
                                "(dc p) col -> p dc col", p=128
                            ),
                        )
                    xts[st] = t

                def proj_rope(c, w_ap_fn, dest_fn):
                    """One 512-col projection chunk + RoPE; dest_fn(raw, tmp)
                    emits the final add(s)."""
                    raw = work.tile([128, CS], bf16, name="raw", tag="raw", bufs=2)
                    tmp = work.tile([128, CS], bf16, name="tmp", tag="tmp", bufs=2)
                    pq = proj_ps()
                    for dc in range(DC):
                        nc.tensor.matmul(
                            pq, w_ap_fn(dc), xts[c][:, dc, :],
                            start=(dc == 0), stop=(dc == DC - 1),
                        )
                    nc.scalar.copy(out=raw[:], in_=pq)
                    pr = proj_ps()
                    nc.tensor.matmul(pr, rot_sb[:], raw[:],
                                     start=True, stop=True)
                    nc.vector.tensor_mul(
                        tmp[:], pr, sin_sb[:, CS * c: CS * (c + 1)]
                    )
                    nc.vector.tensor_mul(
                        raw[:], raw[:], cos_sb[:, CS * c: CS * (c + 1)]
                    )
                    dest_fn(raw, tmp)

                def k_chunk(c):
                    sl = slice(CS * c, CS * (c + 1))

                    def dest(raw, tmp):
                        # aligned halves direct; shifted halves via SBUF DMA
                        nc.vector.tensor_add(
                            kTd[0][0:64, sl], raw[0:64, :], tmp[0:64, :]
                        )
                        nc.vector.tensor_add(
                            kTd[1][64:128, sl], raw[64:128, :], tmp[64:128, :]
                        )
                        nc.scalar.dma_start(
                            out=kTd[0][64:128, sl], in_=kTd[0][0:64, sl]
                        )
                        nc.scalar.dma_start(
                            out=kTd[1][0:64, sl], in_=kTd[1][64:128, sl]
                        )

                    proj_rope(c, lambda dc: wk_sb[:, dc, :], dest)

                def v_chunk(c):
                    vTc = work.tile([128, CS], bf16, name="vT", tag="vT", bufs=2)
                    pq = proj_ps()
                    for dc in range(DC):
                        nc.tensor.matmul(
                            pq, wv_sb[:, dc, :], xts[c][:, dc, :],
                            start=(dc == 0), stop=(dc == DC - 1),
                        )
                    nc.scalar.copy(out=vTc[:], in_=pq)
                    for k4 in range(4):
                        kb = 4 * c + k4
                        ptr = psT.tile([128, 1024], bf16, name="ptr", tag="ptr")
                        nc.tensor.matmul(
                            ptr[:, 0:128], vTc[:, 128 * k4: 128 * (k4 + 1)],
                            idn_sb[:], start=True, stop=True, is_transpose=True,
                        )
                        nc.vector.tensor_copy(
                            out=v_sb[kb][:, 0:64], in_=ptr[:, 0:64]
                        )
                        nc.vector.tensor_copy(
                            out=v_sb[kb][:, 65:129], in_=ptr[:, 64:128]
                        )

                def q_chunk(c, pidx):
                    qt = stream.tile([128, CS], bf16, name=f"qT{pidx}",
                                     tag=f"qT{pidx}", bufs=2)

                    def dest(raw, tmp):
                        nc.vector.tensor_add(qt[:], raw[:], tmp[:])

                    proj_rope(
                        c, lambda dc: wq_sb[:, dc, 128 * pidx: 128 * (pidx + 1)],
                        dest,
                    )
                    return qt

                def oproj_store(qb, rb, po):
                    otb = work.tile([128, DOUT], bf16, name="otb",
                                    tag="otb", bufs=2)
                    nc.vector.tensor_copy(out=otb[:], in_=po[:])
                    nc.sync.dma_start(
                        out=out_d[
                            CS * qb + 128 * rb: CS * qb + 128 * (rb + 1), :
                        ],
                        in_=otb[:],
                    )

                def oproj_steps(qb, cctp):
                    """Yield closures, each emitting one o_proj unit for query
                    block qb (psum alloc / matmul / drain+store per rb).
                    Global pair hc lives in cctp[hc % 4] at dim1 index hc//4."""
                    for rb in range(4):
                        po = [None]

                        def start_rb(po=po):
                            po[0] = psO.tile([128, DOUT], f32, name="po", tag="po")

                        yield start_rb
                        order = [4 * p + g for p in range(4) for g in range(4)]
                        for n, hc in enumerate([4 * g + p for p in range(4)
                                                for g in range(4)]):
                            def mm(rb=rb, hc=hc, n=n, po=po):
                                nc.tensor.matmul(
                                    po[0][:],
                                    cctp[hc % 4][:, hc // 4,
                                                 128 * rb: 128 * (rb + 1)],
                                    wo_sb[:, hc, :],
                                    start=(n == 0), stop=(n == HC - 1),
                                )

                            yield mm

                        def finish_rb(qb=qb, rb=rb, po=po):
                            oproj_store(qb, rb, po[0])

                        yield finish_rb

                def oproj_tail(qb, cctp):
                    """Pair-major o_proj for the last block: accumulate pairs
                    0-2 as their AllGathers land (into psums living in the
                    freed attention psA slots), then finish pair 3 rb-by-rb so
                    stores overlap the remaining matmuls."""
                    po = [proj_ps() for _ in range(4)]
                    for p in range(3):
                        for rb in range(4):
                            for g in range(4):
                                nc.tensor.matmul(
                                    po[rb],
                                    cctp[p][:, g, 128 * rb: 128 * (rb + 1)],
                                    wo_sb[:, 4 * g + p, :],
                                    start=(p == 0 and g == 0), stop=False,
                                )
                    for rb in range(4):
                        for g in range(4):
                            nc.tensor.matmul(
                                po[rb],
                                cctp[3][:, g, 128 * rb: 128 * (rb + 1)],
                                wo_sb[:, 4 * g + 3, :],
                                start=False, stop=(g == 3),
                            )
                        oproj_store(qb, rb, po[rb])

                def attn_round(c, qts, filler):
                    kmax = 4 * (c + 1)
                    cctp = []

                    def fill(n):
                        for _ in range(n):
                            f = next(filler, None)
                            if f is None:
                                return
                            f()

                    for pidx in range(4):
                        hg = pidx // 2
                        qt = qts[pidx]
                        pav = psAV.tile([65, 1024], f32, name="pav", tag="pav")
                        pending = []

                        def emit_av(kb, vw, pt, pav=pav, kmax=kmax):
                            for i in range(2):
                                nc.tensor.matmul(
                                    pav[:, 512 * i + 512 - vw: 512 * (i + 1)],
                                    v_sb[kb][:, 65 * hg: 65 * hg + 65],
                                    pt[:, 512 * i: 512 * i + vw],
                                    start=(kb == 0), stop=(kb == kmax - 1),
                                )

                        fill(1)
                        for kb in range(kmax):
                            j = kb - 4 * c
                            vw = 512 - 128 * j if j >= 1 else 512
                            ps = psA.tile([128, 1024], f32, name="ps", tag="ps")
                            for i in range(2):
                                r0 = 64 * i
                                nc.tensor.matmul(
                                    ps[:, 512 * i: 512 * i + vw],
                                    kTd[hg][r0: r0 + 64, 128 * kb: 128 * (kb + 1)],
                                    qt[r0: r0 + 64, 512 - vw: 512],
                                    start=True, stop=True,
                                )
                            pt = work.tile([128, 1024], bf16, name="pt", tag="pt",
                                           bufs=4)
                            if vw == 512:
                                nc.scalar.activation(
                                    out=pt[:], in_=ps[:], func=Exp, scale=0.125
                                )
                                if j >= 0:
                                    nc.vector.tensor_mul(
                                        pt[:], pt[:], msk_sb[:, j, :]
                                    )
                            else:
                                ptw = pt[:, :].rearrange(
                                    "p (i c) -> p i c", i=2)[:, :, 0:vw]
                                psw = ps[:, :].rearrange(
                                    "p (i c) -> p i c", i=2)[:, :, 0:vw]
                                mskw = msk_sb[:, 0, :].rearrange(
                                    "p (i c) -> p i c", i=2)[:, :, 0:vw]
                                nc.scalar.activation(
                                    out=ptw, in_=psw, func=Exp, scale=0.125
                                )
                                nc.vector.tensor_mul(ptw, ptw, mskw)
                            pending.append((kb, vw, pt))
                            if len(pending) > 3:
                                emit_av(*pending.pop(0))
                            fill(1)
                        while pending:
                            emit_av(*pending.pop(0))
                        fill(4)
                        # drain AV psum fast (frees pav for the next pair):
                        # value rows via DVE copy, sum row via bf16 copy; then
                        # PE-broadcast the sums, reciprocal, scale.
                        sumb = work.tile([65, 1024], bf16, name="sumb", tag="sumb",
                                         bufs=2)
                        nc.scalar.copy(out=sumb[64:65, :], in_=pav[64:65, :])
                        pavc = work.tile([65, 1024], f32, name="pavc", tag="pavc",
                                         bufs=2)
                        nc.vector.tensor_copy(out=pavc[0:64, :], in_=pav[0:64, :])
                        rbc = work.tile([64, 1024], f32, name="rbc", tag="rbc",
                                        bufs=2)
                        for i in range(2):
                            pbt = psT.tile([128, 1024], bf16, name="pb", tag="ptr")
                            pb = pbt[0:64, 0:1024].bitcast(f32)
                            nc.tensor.matmul(
                                pb, ones_sb[64:65, :],
                                sumb[64:65, 512 * i: 512 * (i + 1)],
                                start=True, stop=True,
                            )
                            nc.vector.reciprocal_approx_fast(
                                out=rbc[:, 512 * i: 512 * (i + 1)], in_=pb
                            )
                        at = work.tile([64, 1024], bf16, name="at", tag="at",
                                       bufs=2)
                        nc.vector.tensor_mul(at[:], pavc[0:64, :], rbc[:])
                        # at[p, 512i+col] -> cc_in rows 64i+p
                        nc.sync.dma_start(
                            out=cc_in[c][pidx][:, :].rearrange(
                                "(i p) col -> p i col", i=2
                            ),
                            in_=at[:, :].rearrange("p (i col) -> p i col", i=2),
                        )
                        if NO_CC:
                            nc.sync.dma_start(
                                out=cc_out[c][pidx][0:128, :],
                                in_=cc_in[c][pidx][:, :],
                            )
                        else:
                            nc.gpsimd.collective_compute(
                                "AllGather",
                                mybir.AluOpType.bypass,
                                replica_groups=RG,
                                ins=[cc_in[c][pidx].opt()],
                                outs=[cc_out[c][pidx].opt()],
                            )
                        cp = work.tile([128, 4, 512], bf16, name=f"cct{pidx}",
                                       tag=f"cct{pidx}", bufs=2)
                        nc.sync.dma_start(
                            out=cp[:],
                            in_=cc_out[c][pidx][:, :].rearrange(
                                "(g p) col -> p g col", p=128
                            ),
                        )
                        cctp.append(cp)
                    fill(100)
                    return cctp

                # ---------------- rounds ----------------
                load_stage(0, split=4)
                nc.sync.dma_start(out=cos_sb[:], in_=cos_d[:, :])
                nc.sync.dma_start(out=sin_sb[:], in_=sin_d[:, :])
                nc.sync.dma_start(out=wv_sb[:], in_=wv_d[:, :])
                nc.sync.dma_start(out=rot_sb[:], in_=rot_d[:, :])
                nc.sync.dma_start(out=idn_sb[:], in_=idn_d[:, :])
                nc.sync.dma_start(out=wq_sb[:], in_=wq_d[:, :])
                nc.sync.dma_start(out=msk_sb[:], in_=msk_d[:, :, :])
                load_stage(1)
                nc.sync.dma_start(out=wo_sb[:], in_=wo_d[:, :])
                ccts = {}
                for c in range(NCH):
                    if 2 <= c + 1 < NCH:
                        load_stage(c + 1)
                    k_chunk(c)
                    v_chunk(c)
                    qts = [q_chunk(c, p) for p in range(4)]
                    filler = oproj_steps(c - 1, ccts[c - 1]) if c >= 1 else iter(())
                    ccts[c] = attn_round(c, qts, filler)
                oproj_tail(NCH - 1, ccts[NCH - 1])

            if bench_iters:
                with tc.For_i(0, bench_iters, 1, name="bench"):
                    emit_body()
            else:
                emit_body()

    nc.compile()
    return nc


def prep_inputs(x, cos, sin, wq, wk, wv, wo):
    """Shard + reformat full inputs into per-core input maps."""
    bf = ml_dtypes.bfloat16
    b, s, d = x.shape
    dout = d // 4
    dc = d // 128
    cos2 = np.tile(np.ascontiguousarray(cos.T), (2, 1)).astype(bf)
    sinT = np.ascontiguousarray(sin.T)
    sinsw = np.concatenate([-sinT[:32], sinT[32:]], axis=0)
    sinsw2 = np.tile(sinsw, (2, 1)).astype(bf)
    # rotate-half permutation: tmp[i] = raw[sigma(i)]; out = R.T @ raw
    rotm = np.zeros((128, 128), np.float32)
    for i in range(128):
        j = (i // 64) * 64 + ((i % 64) + 32) % 64
        rotm[j, i] = 1.0
    rotm = rotm.astype(bf)
    ident = np.eye(128, dtype=np.float32).astype(bf)
    k_loc = np.arange(128)[:, None]
    q_loc = np.arange(512)[None, :]
    ms = []
    for j in range(4):
        mj = (k_loc <= q_loc - 128 * j).astype(np.float32)
        ms.append(np.concatenate([mj, mj], axis=1))
    masks = np.stack(ms, axis=1).astype(bf)  # [128, 4, 1024]

    def pack_w(w):  # [d, cols] -> [128, dc*cols] with w rows 128-blocked
        cols = w.shape[1]
        return np.ascontiguousarray(
            w.reshape(dc, 128, cols).transpose(1, 0, 2).reshape(128, dc * cols)
        ).astype(bf)

    in_maps = []
    for c in range(N_CORES):
        bb, g = divmod(c, 4)
        in_maps.append(
            {
                "xT": np.ascontiguousarray(x[bb].T).astype(bf),
                "wqt": pack_w(wq[:, 512 * g: 512 * (g + 1)]),
                "wkt": pack_w(wk[:, 128 * g: 128 * (g + 1)]),
                "wvt": pack_w(wv[:, 128 * g: 128 * (g + 1)]),
                "wot": pack_w(wo[:, dout * g: dout * (g + 1)]),
                "cos2": cos2,
                "sinsw2": sinsw2,
                "rot": rotm,
                "ident": ident,
                "masks": masks,
            }
        )
    return in_maps


def assemble_output(results, b, s, d):
    full = np.empty((b, s, d), np.float32)
    dout = d // 4
    for c in range(N_CORES):
        bb, g = divmod(c, 4)
        full[bb][:, dout * g: dout * (g + 1)] = results[c]["out"].astype(np.float32)
    return full


def kernel(**inputs):
    x = np.asarray(inputs["x"], np.float32)
    b, s, d = x.shape
    key = (s, d)
    if key not in _cache:
        _cache[key] = build_program(S=s, D=d)
    nc = _cache[key]
    in_maps = prep_inputs(
        x,
        np.asarray(inputs["cos"], np.float32),
        np.asarray(inputs["sin"], np.float32),
        np.asarray(inputs["wq"], np.float32),
        np.asarray(inputs["wk"], np.float32),
        np.asarray(inputs["wv"], np.float32),
        np.asarray(inputs["wo"], np.float32),
    )
    from concourse.bass_utils import run_bass_kernel_spmd

    res = run_bass_kernel_spmd(nc, in_maps, core_ids=list(range(N_CORES)))
    return assemble_output(res.results, b, s, d)


# revision 33
# speedup vs baseline: 1.1745x; 1.1745x over previous
"""Trainium2 Bass kernel for GQA attention (nn_Attention_15015205667492).

Reference computation (per batch b, seq s=2048, d=2048):
  q = (x @ wq)  -> 32 heads x 64     (RoPE)
  k = (x @ wk)  ->  8 kv heads x 64  (RoPE)
  v = (x @ wv)  ->  8 kv heads x 64
  causal softmax(q k^T / 8) @ v  (GQA: kv head = q head // 4)
  out = attn @ wo
Sharding (8 cores): DP2 x TP4.
  core c: batch = c//4, head-group g = c%4 (Q heads 8g..8g+7, KV heads 2g, 2g+1).
  Per-qb attention output (head-transposed, bf16) is AllGathered within the
  4-core batch group; each core then computes a 512-column slice of o_proj so
  per-core outputs are disjoint blocks of the final output.

Schedule: 4 rounds, one per 512-column sequence chunk.  Round c streams x
stage c+1 (single rearranged HWDGE DMA), projects K/V/Q for chunk c, runs
attention for query block c, and interleaves o_proj matmuls of block c-1
into the attention stream (fills PE gaps while ScalarE computes exp; AV
matmuls lag QK by 3 blocks so the exp(+mask on diagonal blocks) latency is
hidden).  The AllGather is split per head-pair (16 x 128KB) so the last
block's o_proj can start accumulating pair-by-pair as gathers land; the
final block runs pair-major in the freed attention psum banks.  V is
projected transposed and flipped per 128-block with PE transpose matmuls
(~5x fewer PE instructions than the natural-layout projection).  Scores
stay transposed (S^T[k,q]); causal key blocks shrink their query window
(vw = 512-128j) and diagonal blocks use one strided-AP exp/mask pair.  The
AV psum's 65th row (ones column in V) accumulates softmax denominators,
broadcast with a 1x64 bf16 matmul and inverted with
reciprocal_approx_fast.  Drains are spread over DVE/ScalarE; DMAs ride the
SP/ACT HWDGE queues (gpsimd SWDGE is ~1us/DMA); output is written bf16 and
widened on the host.
"""

import sys

sys.path.insert(0, "/opt/trn_rl_repo")

import numpy as np
import ml_dtypes

N_CORES = 8
H, KVH, HD = 32, 8, 64
RG = [[0, 1, 2, 3], [4, 5, 6, 7]]

_cache = {}


def build_program(S=2048, D=2048, enable_asserts=False, NO_CC=False, bench_iters=0,
                  phases=None, ablate=(), fuse_exp=True, pt_bufs=4, av_lag=3,
                  vw_j1=True):
    import concourse.mybir as mybir
    import concourse.tile as tile
    from concourse import bacc

    f32 = mybir.dt.float32
    bf16 = mybir.dt.bfloat16
    Exp = mybir.ActivationFunctionType.Exp

    DC = D // 128         # contraction chunks for projections (16)
    CS = 512              # seq chunk = query block
    NCH = S // CS         # rounds (4)
    DOUT = D // 4         # output column slice per core (512)
    HC = (H * HD) // 128  # o_proj contraction chunks (16)
    KB = S // 128         # key blocks (16)

    nc = bacc.Bacc(
        "TRN2",
        target_bir_lowering=False,
        debug=False,
        enable_asserts=enable_asserts,
        num_devices=N_CORES,
    )

    xT_d = nc.dram_tensor("xT", [D, S], bf16, kind="ExternalInput")
    wk_d = nc.dram_tensor("wkt", [128, DC * 128], bf16, kind="ExternalInput")
    wv_d = nc.dram_tensor("wvt", [128, DC * 128], bf16, kind="ExternalInput")
    wq_d = nc.dram_tensor("wqt", [128, DC * 512], bf16, kind="ExternalInput")
    wo_d = nc.dram_tensor("wot", [128, HC * DOUT], bf16, kind="ExternalInput")
    cos_d = nc.dram_tensor("cos2", [128, S], bf16, kind="ExternalInput")
    sin_d = nc.dram_tensor("sinsw2", [128, S], bf16, kind="ExternalInput")
    rot_d = nc.dram_tensor("rot", [128, 128], bf16, kind="ExternalInput")
    idn_d = nc.dram_tensor("ident", [128, 128], bf16, kind="ExternalInput")
    msk_d = nc.dram_tensor("masks", [128, 4, 1024], bf16, kind="ExternalInput")
    out_d = nc.dram_tensor("out", [S, DOUT], bf16, kind="ExternalOutput")

    with tile.TileContext(nc) as tc:
        with (
            tc.tile_pool(name="const", bufs=1) as const,
            tc.tile_pool(name="stream", bufs=2) as stream,
            tc.tile_pool(name="work", bufs=2) as work,
            tc.tile_pool(name="psA", bufs=2, space="PSUM") as psA,
            tc.tile_pool(name="psAV", bufs=1, space="PSUM") as psAV,
            tc.tile_pool(name="psO", bufs=1, space="PSUM") as psO,
            tc.tile_pool(name="psT", bufs=1, space="PSUM") as psT,
            tc.tile_pool(name="dram", bufs=1, space="DRAM") as dram,
        ):
            # ---------------- constants (order = SP-queue DMA order) --------
            wk_sb = const.tile([128, DC, 128], bf16, name="wk", tag="wk")
            nc.sync.dma_start(out=wk_sb[:], in_=wk_d[:, :])
            wv_sb = const.tile([128, DC, 128], bf16, name="wv", tag="wv")
            cos_sb = const.tile([128, S], bf16, name="cos", tag="cos")
            sin_sb = const.tile([128, S], bf16, name="sin", tag="sin")
            rot_sb = const.tile([128, 128], bf16, name="rot", tag="rot")
            idn_sb = const.tile([128, 128], bf16, name="idn", tag="idn")
            wq_sb = const.tile([128, DC, 512], bf16, name="wq", tag="wq")
            msk_sb = const.tile([128, 4, 1024], bf16, name="msk", tag="msk")
            wo_sb = const.tile([128, HC, DOUT], bf16, name="wo", tag="wo")

            ones_sb = const.tile([65, 64], bf16, name="ones", tag="ones")
            nc.vector.memset(ones_sb[:], 1.0)
            kTd = [
                const.tile([128, S], bf16, name=f"kTd{h}", tag=f"kTd{h}")
                for h in range(2)
            ]
            v_sb = []
            for kb in range(KB):
                vt = const.tile([128, 132], bf16, name=f"v{kb}", tag=f"v{kb}")
                nc.vector.memset(vt[:, 64:65], 1.0)
                nc.vector.memset(vt[:, 129:130], 1.0)
                v_sb.append(vt)

            # per-(block, pair) collective buffers: pair p's AllGather output
            # rows 128*g hold global head-pair hc = 4*g + p
            cc_in = [
                [dram.tile([128, 512], bf16, name=f"cin{qb}_{p}",
                           tag=f"cin{qb}_{p}") for p in range(4)]
                for qb in range(NCH)
            ]
            cc_out = [
                [dram.tile([512, 512], bf16, name=f"cout{qb}_{p}",
                           tag=f"cout{qb}_{p}") for p in range(4)]
                for qb in range(NCH)
            ]

            def emit_body():
                xts = [None] * NCH
                pstate = {"n": 0, "tile": None}

                def proj_ps():
                    h = pstate["n"] % 2
                    if h == 0:
                        pstate["tile"] = psA.tile([128, 1024], f32, name="pjp",
                                                  tag="ps")
                    pstate["n"] += 1
                    return pstate["tile"][:, 512 * h: 512 * (h + 1)]

                def load_stage(st, split=1):
                    t = stream.tile([128, DC, CS], bf16, name="xs", tag="xs",
                                    bufs=2)
                    q = DC // split
                    for i in range(split):
                        nc.sync.dma_start(
                            out=t[:, q * i: q * (i + 1), :],
                            in_=xT_d[128 * q * i: 128 * q * (i + 1),
                                     CS * st: CS * (st + 1)].rearrange(
                                "(dc p) col -> p dc col", p=128
                            ),
                        )
                    xts[st] = t

                def proj_rope(c, w_ap_fn, dest_fn):
                    """One 512-col projection chunk + RoPE; dest_fn(raw, tmp)
                    emits the final add(s)."""
                    raw = work.tile([128, CS], bf16, name="raw", tag="raw", bufs=2)
                    tmp = work.tile([128, CS], bf16, name="tmp", tag="tmp", bufs=2)
                    pq = proj_ps()
                    for dc in range(DC):
                        nc.tensor.matmul(
                            pq, w_ap_fn(dc), xts[c][:, dc, :],
                            start=(dc == 0), stop=(dc == DC - 1),
                        )
                    nc.scalar.copy(out=raw[:], in_=pq)
                    pr = proj_ps()
                    nc.tensor.matmul(pr, rot_sb[:], raw[:],
                                     start=True, stop=True)
                    nc.vector.tensor_mul(
                        tmp[:], pr, sin_sb[:, CS * c: CS * (c + 1)]
                    )
                    nc.vector.tensor_mul(
                        raw[:], raw[:], cos_sb[:, CS * c: CS * (c + 1)]
                    )
                    dest_fn(raw, tmp)

                def k_chunk(c):
                    sl = slice(CS * c, CS * (c + 1))

                    def dest(raw, tmp):
                        # aligned halves direct; shifted halves via SBUF DMA
                        nc.vector.tensor_add(
                            kTd[0][0:64, sl], raw[0:64, :], tmp[0:64, :]
                        )
                        nc.vector.tensor_add(
                            kTd[1][64:128, sl], raw[64:128, :], tmp[64:128, :]
                        )
                        nc.scalar.dma_start(
                            out=kTd[0][64:128, sl], in_=kTd[0][0:64, sl]
                        )
                        nc.scalar.dma_start(
                            out=kTd[1][0:64, sl], in_=kTd[1][64:128, sl]
                        )

                    proj_rope(c, lambda dc: wk_sb[:, dc, :], dest)

                def v_chunk(c):
                    vTc = work.tile([128, CS], bf16, name="vT", tag="vT", bufs=2)
                    pq = proj_ps()
                    for dc in range(DC):
                        nc.tensor.matmul(
                            pq, wv_sb[:, dc, :], xts[c][:, dc, :],
                            start=(dc == 0), stop=(dc == DC - 1),
                        )
                    nc.scalar.copy(out=vTc[:], in_=pq)
                    for k4 in range(4):
                        kb = 4 * c + k4
                        ptr = psT.tile([128, 1024], bf16, name="ptr", tag="ptr")
                        nc.tensor.matmul(
                            ptr[:, 0:128], vTc[:, 128 * k4: 128 * (k4 + 1)],
                            idn_sb[:], start=True, stop=True, is_transpose=True,
                        )
                        nc.vector.tensor_copy(
                            out=v_sb[kb][:, 0:64], in_=ptr[:, 0:64]
                        )
                        nc.vector.tensor_copy(
                            out=v_sb[kb][:, 65:129], in_=ptr[:, 64:128]
                        )

                def q_chunk(c, pidx):
                    qt = stream.tile([128, CS], bf16, name=f"qT{pidx}",
                                     tag=f"qT{pidx}", bufs=2)

                    def dest(raw, tmp):
                        nc.vector.tensor_add(qt[:], raw[:], tmp[:])

                    proj_rope(
                        c, lambda dc: wq_sb[:, dc, 128 * pidx: 128 * (pidx + 1)],
                        dest,
                    )
                    return qt

                def oproj_store(qb, rb, po):
                    otb = work.tile([128, DOUT], bf16, name="otb",
                                    tag="otb", bufs=2)
                    nc.vector.tensor_copy(out=otb[:], in_=po[:])
                    nc.sync.dma_start(
                        out=out_d[
                            CS * qb + 128 * rb: CS * qb + 128 * (rb + 1), :
                        ],
                        in_=otb[:],
                    )

                def oproj_steps(qb, cctp):
                    """Yield closures, each emitting one o_proj unit for query
                    block qb (psum alloc / matmul / drain+store per rb).
                    Global pair hc lives in cctp[hc % 4] at dim1 index hc//4."""
                    for rb in range(4):
                        po = [None]

                        def start_rb(po=po):
                            po[0] = psO.tile([128, DOUT], f32, name="po", tag="po")

                        yield start_rb
                        order = [4 * p + g for p in range(4) for g in range(4)]
                        for n, hc in enumerate([4 * g + p for p in range(4)
                                                for g in range(4)]):
                            def mm(rb=rb, hc=hc, n=n, po=po):
                                nc.tensor.matmul(
                                    po[0][:],
                                    cctp[hc % 4][:, hc // 4,
                                                 128 * rb: 128 * (rb + 1)],
                                    wo_sb[:, hc, :],
                                    start=(n == 0), stop=(n == HC - 1),
                                )

                            yield mm

                        def finish_rb(qb=qb, rb=rb, po=po):
                            oproj_store(qb, rb, po[0])

                        yield finish_rb

                def oproj_tail(qb, cctp):
                    """Pair-major o_proj for the last block: accumulate pairs
                    0-2 as their AllGathers land (into psums living in the
                    freed attention psA slots), then finish pair 3 rb-by-rb so
                    stores overlap the remaining matmuls."""
                    po = [proj_ps() for _ in range(4)]
                    for p in range(3):
                        for rb in range(4):
                            for g in range(4):
                                nc.tensor.matmul(
                                    po[rb],
                                    cctp[p][:, g, 128 * rb: 128 * (rb + 1)],
                                    wo_sb[:, 4 * g + p, :],
                                    start=(p == 0 and g == 0), stop=False,
                                )
                    for rb in range(4):
                        for g in range(4):
                            nc.tensor.matmul(
                                po[rb],
                                cctp[3][:, g, 128 * rb: 128 * (rb + 1)],
                                wo_sb[:, 4 * g + 3, :],
                                start=False, stop=(g == 3),
                            )
                        oproj_store(qb, rb, po[rb])

                def attn_round(c, qts, filler):
                    kmax = 4 * (c + 1)
                    cctp = []

                    def fill(n):
                        for _ in range(n):
                            f = next(filler, None)
                            if f is None:
                                return
                            f()

                    for pidx in range(4):
                        hg = pidx // 2
                        qt = qts[pidx]
                        pav = psAV.tile([65, 1024], f32, name="pav", tag="pav")
                        pending = []

                        def emit_av(kb, vw, pt, pav=pav, kmax=kmax):
                            for i in range(2):
                                nc.tensor.matmul(
                                    pav[:, 512 * i + 512 - vw: 512 * (i + 1)],
                                    v_sb[kb][:, 65 * hg: 65 * hg + 65],
                                    pt[:, 512 * i: 512 * i + vw],
                                    start=(kb == 0), stop=(kb == kmax - 1),
                                )

                        fill(1)
                        for kb in range(kmax):
                            j = kb - 4 * c
                            vw = 512 - 128 * j if j >= (1 if vw_j1 else 2) else 512
                            ps = psA.tile([128, 1024], f32, name="ps", tag="ps")
                            for i in range(2):
                                r0 = 64 * i
                                nc.tensor.matmul(
                                    ps[:, 512 * i: 512 * i + vw],
                                    kTd[hg][r0: r0 + 64, 128 * kb: 128 * (kb + 1)],
                                    qt[r0: r0 + 64, 512 - vw: 512],
                                    start=True, stop=True,
                                )
                            pt = work.tile([128, 1024], bf16, name="pt", tag="pt",
                                           bufs=pt_bufs)
                            if vw == 512:
                                nc.scalar.activation(
                                    out=pt[:], in_=ps[:], func=Exp, scale=0.125
                                )
                                if j >= 0:
                                    nc.vector.tensor_mul(
                                        pt[:], pt[:], msk_sb[:, j, :]
                                    )
                            elif fuse_exp:
                                ptw = pt[:, :].rearrange(
                                    "p (i c) -> p i c", i=2)[:, :, 0:vw]
                                psw = ps[:, :].rearrange(
                                    "p (i c) -> p i c", i=2)[:, :, 0:vw]
                                mskw = msk_sb[:, 0, :].rearrange(
                                    "p (i c) -> p i c", i=2)[:, :, 0:vw]
                                nc.scalar.activation(
                                    out=ptw, in_=psw, func=Exp, scale=0.125
                                )
                                nc.vector.tensor_mul(ptw, ptw, mskw)
                            else:
                                for i in range(2):
                                    sl = slice(512 * i, 512 * i + vw)
                                    nc.scalar.activation(
                                        out=pt[:, sl], in_=ps[:, sl], func=Exp,
                                        scale=0.125,
                                    )
                                    nc.vector.tensor_mul(
                                        pt[:, sl], pt[:, sl], msk_sb[:, 0, 0:vw]
                                    )
                            pending.append((kb, vw, pt))
                            if len(pending) > av_lag:
                                emit_av(*pending.pop(0))
                            fill(1)
                        while pending:
                            emit_av(*pending.pop(0))
                        fill(4)
                        # drain AV psum fast (frees pav for the next pair):
                        # value rows via DVE copy, sum row via bf16 copy; then
                        # PE-broadcast the sums, reciprocal, scale.
                        sumb = work.tile([65, 1024], bf16, name="sumb", tag="sumb",
                                         bufs=2)
                        nc.scalar.copy(out=sumb[64:65, :], in_=pav[64:65, :])
                        pavc = work.tile([65, 1024], f32, name="pavc", tag="pavc",
                                         bufs=2)
                        nc.vector.tensor_copy(out=pavc[0:64, :], in_=pav[0:64, :])
                        rbc = work.tile([64, 1024], f32, name="rbc", tag="rbc",
                                        bufs=2)
                        for i in range(2):
                            pbt = psT.tile([128, 1024], bf16, name="pb", tag="ptr")
                            pb = pbt[0:64, 0:1024].bitcast(f32)
                            nc.tensor.matmul(
                                pb, ones_sb[64:65, :],
                                sumb[64:65, 512 * i: 512 * (i + 1)],
                                start=True, stop=True,
                            )
                            nc.vector.reciprocal_approx_fast(
                                out=rbc[:, 512 * i: 512 * (i + 1)], in_=pb
                            )
                        at = work.tile([64, 1024], bf16, name="at", tag="at",
                                       bufs=2)
                        nc.vector.tensor_mul(at[:], pavc[0:64, :], rbc[:])
                        # at[p, 512i+col] -> cc_in rows 64i+p
                        nc.sync.dma_start(
                            out=cc_in[c][pidx][:, :].rearrange(
                                "(i p) col -> p i col", i=2
                            ),
                            in_=at[:, :].rearrange("p (i col) -> p i col", i=2),
                        )
                        if NO_CC:
                            nc.sync.dma_start(
                                out=cc_out[c][pidx][0:128, :],
                                in_=cc_in[c][pidx][:, :],
                            )
                        else:
                            nc.gpsimd.collective_compute(
                                "AllGather",
                                mybir.AluOpType.bypass,
                                replica_groups=RG,
                                ins=[cc_in[c][pidx].opt()],
                                outs=[cc_out[c][pidx].opt()],
                            )
                        cp = work.tile([128, 4, 512], bf16, name=f"cct{pidx}",
                                       tag=f"cct{pidx}", bufs=2)
                        nc.sync.dma_start(
                            out=cp[:],
                            in_=cc_out[c][pidx][:, :].rearrange(
                                "(g p) col -> p g col", p=128
                            ),
                        )
                        cctp.append(cp)
                    fill(100)
                    return cctp

                # ---------------- rounds ----------------
                load_stage(0, split=4)
                nc.sync.dma_start(out=cos_sb[:], in_=cos_d[:, :])
                nc.sync.dma_start(out=sin_sb[:], in_=sin_d[:, :])
                nc.sync.dma_start(out=wv_sb[:], in_=wv_d[:, :])
                nc.sync.dma_start(out=rot_sb[:], in_=rot_d[:, :])
                nc.sync.dma_start(out=idn_sb[:], in_=idn_d[:, :])
                nc.sync.dma_start(out=wq_sb[:], in_=wq_d[:, :])
                nc.sync.dma_start(out=msk_sb[:], in_=msk_d[:, :, :])
                load_stage(1)
                nc.sync.dma_start(out=wo_sb[:], in_=wo_d[:, :])
                ccts = {}
                for c in range(NCH):
                    if 2 <= c + 1 < NCH:
                        load_stage(c + 1)
                    k_chunk(c)
                    v_chunk(c)
                    qts = [q_chunk(c, p) for p in range(4)]
                    filler = oproj_steps(c - 1, ccts[c - 1]) if c >= 1 else iter(())
                    ccts[c] = attn_round(c, qts, filler)
                oproj_tail(NCH - 1, ccts[NCH - 1])

            if bench_iters:
                with tc.For_i(0, bench_iters, 1, name="bench"):
                    emit_body()
            else:
                emit_body()

    nc.compile()
    return nc


def prep_inputs(x, cos, sin, wq, wk, wv, wo):
    """Shard + reformat full inputs into per-core input maps."""
    bf = ml_dtypes.bfloat16
    b, s, d = x.shape
    dout = d // 4
    dc = d // 128
    cos2 = np.tile(np.ascontiguousarray(cos.T), (2, 1)).astype(bf)
    sinT = np.ascontiguousarray(sin.T)
    sinsw = np.concatenate([-sinT[:32], sinT[32:]], axis=0)
    sinsw2 = np.tile(sinsw, (2, 1)).astype(bf)
    # rotate-half permutation: tmp[i] = raw[sigma(i)]; out = R.T @ raw
    rotm = np.zeros((128, 128), np.float32)
    for i in range(128):
        j = (i // 64) * 64 + ((i % 64) + 32) % 64
        rotm[j, i] = 1.0
    rotm = rotm.astype(bf)
    ident = np.eye(128, dtype=np.float32).astype(bf)
    k_loc = np.arange(128)[:, None]
    q_loc = np.arange(512)[None, :]
    ms = []
    for j in range(4):
        mj = (k_loc <= q_loc - 128 * j).astype(np.float32)
        ms.append(np.concatenate([mj, mj], axis=1))
    masks = np.stack(ms, axis=1).astype(bf)  # [128, 4, 1024]

    def pack_w(w):  # [d, cols] -> [128, dc*cols] with w rows 128-blocked
        cols = w.shape[1]
        return np.ascontiguousarray(
            w.reshape(dc, 128, cols).transpose(1, 0, 2).reshape(128, dc * cols)
        ).astype(bf)

    in_maps = []
    for c in range(N_CORES):
        bb, g = divmod(c, 4)
        in_maps.append(
            {
                "xT": np.ascontiguousarray(x[bb].T).astype(bf),
                "wqt": pack_w(wq[:, 512 * g: 512 * (g + 1)]),
                "wkt": pack_w(wk[:, 128 * g: 128 * (g + 1)]),
                "wvt": pack_w(wv[:, 128 * g: 128 * (g + 1)]),
                "wot": pack_w(wo[:, dout * g: dout * (g + 1)]),
                "cos2": cos2,
                "sinsw2": sinsw2,
                "rot": rotm,
                "ident": ident,
                "masks": masks,
            }
        )
    return in_maps


def assemble_output(results, b, s, d):
    full = np.empty((b, s, d), np.float32)
    dout = d // 4
    for c in range(N_CORES):
        bb, g = divmod(c, 4)
        full[bb][:, dout * g: dout * (g + 1)] = results[c]["out"].astype(np.float32)
    return full


def kernel(**inputs):
    x = np.asarray(inputs["x"], np.float32)
    b, s, d = x.shape
    key = (s, d)
    if key not in _cache:
        _cache[key] = build_program(S=s, D=d)
    nc = _cache[key]
    in_maps = prep_inputs(
        x,
        np.asarray(inputs["cos"], np.float32),
        np.asarray(inputs["sin"], np.float32),
        np.asarray(inputs["wq"], np.float32),
        np.asarray(inputs["wk"], np.float32),
        np.asarray(inputs["wv"], np.float32),
        np.asarray(inputs["wo"], np.float32),
    )
    from concourse.bass_utils import run_bass_kernel_spmd

    res = run_bass_kernel_spmd(nc, in_maps, core_ids=list(range(N_CORES)))
    return assemble_output(res.results, b, s, d)
